# revision 13
# baseline (speedup 1.0000x reference)
"""Causal multi-head self-attention with RoPE on 8 Trainium2 NeuronCores.

Sharding: data-parallel over batch (B=4 -> 2 cores per batch) x tensor-parallel
over heads (16 heads -> 8 per core). Each core computes q/k/v projections for
its 8 heads, RoPE, causal attention, and a partial o_proj; the host sums the
two partial o_proj outputs per batch.

Structure (v3):
  - One fused pipeline: per 512-query chunk sc, the Q/K/V projection chains of
    chunk sc+1 and the o_proj chains of chunk sc-1 are interleaved as PE
    "filler" work inside attention(sc)'s scores/AV stream, so the tensor
    engine never idles at phase boundaries and stays at full p-state clock.
    Fillers are front-loaded at sc=0 (covers the RoPE(0) wait) and reserved
    at each chunk's end (covers the normalization drain).
  - bf16 for qkT/vp/e_t/heads/wo (1 cyc/row at any N, incl. the short
    diagonal tiles that f32r runs at 4 cyc/row); x/Wq/Wk/Wv stay f32r so the
    softmax logits keep ~1e-2 absolute accuracy. The tile_position-packed
    score pairs overlap in the PE row groups (2nd matmul of a pair is ~free).
  - RoPE swap is a PE matmul against a bf16 block-swap permutation; its
    emission is deferred one filler so the PE never waits on the PSUM->SBUF
    cast of its input. (DMA-based swap costs ~600ns of descriptor-gen
    dispatch per transfer -- far worse than the 213ns matmul.)
  - Rotary tables are built on device (Cody-Waite range reduction, two
    1024-col halves); the angle outer product runs on gpsimd+DVE
    (partition_broadcast + tensor_scalar_mul), keeping the cold-clock PE
    out of the table critical path. pos*invf needs exact f32 here --
    f32r rounding of invf turns into O(1) rad angle error at pos ~ 2e3.
  - PSUM budget (8 banks): score pairs [128,1024] x2 (4 banks), work tiles
    [128,512] x2 (proj chains / RoPE swap / o_proj, 2 banks), AV
    accumulators [65,512] x2 (2 banks, drained to SBUF by DVE immediately).
  - exp runs on the scalar engine straight out of PSUM over the packed
    head-pair tile (the attention cadence is exp-bound at ~1.1us/pair; PE
    fillers absorb the slack). Causal masking of the diagonal 128-col block
    is a gpsimd affine_select on e_t.
  - Normalization: DVE reciprocal (via a partition-0 staging copy -- the
    approx-reciprocal sequence cannot take a partition-offset input),
    gpsimd broadcast, DVE multiply writing heads_t directly for the even
    head and via one DMA for the odd head (cross-partition move).
"""

import sys

sys.path.insert(0, "/opt/trn_rl_repo")

import numpy as np

import concourse.bass as bass
import concourse.tile as tile
from concourse import bacc, mybir
from concourse.bass_utils import run_bass_kernel_spmd
from concourse.masks import make_identity

B, S, D, H = 4, 2048, 1024, 16
DK = D // H            # 64
HPC = H // 2           # 8 heads per core
DPC = HPC * DK         # 512 head dims per core
N_CORES = 8
HALF = DK // 2         # 32 rotary pairs
THETA = 10000.0
NKC = S // 128         # 16 key tiles
NSC = S // 512         # 4 query/proj chunks

AF = mybir.ActivationFunctionType
F32 = mybir.dt.float32
F32R = mybir.dt.float32r
BF16 = mybir.dt.bfloat16
I32 = mybir.dt.int32

TWO_PI = 2.0 * np.pi
# 3-term Cody-Waite split of 2*pi (c1/c2 have short mantissas so k*c is exact)
_CW_C1 = 6.28125
_CW_C2 = float(np.float32(9.67025756835937500e-4))
_CW_C3 = float(TWO_PI - _CW_C1 - np.float32(9.67025756835937500e-4))


def _build_program(debug=False):
    nc = bacc.Bacc("TRN2", target_bir_lowering=False, debug=False)

    xT = nc.dram_tensor("xT", [D, S], F32, kind="ExternalInput").ap()
    wqT = nc.dram_tensor("wqT", [D, DPC], F32, kind="ExternalInput").ap()
    wkT = nc.dram_tensor("wkT", [D, DPC], F32, kind="ExternalInput").ap()
    wvT = nc.dram_tensor("wvT", [D, DPC], F32, kind="ExternalInput").ap()
    woT = nc.dram_tensor("woT", [DPC, D], BF16, kind="ExternalInput").ap()
    pos = nc.dram_tensor("pos", [S], I32, kind="ExternalInput").ap()
    invf_in = nc.dram_tensor("invf", [HALF], F32, kind="ExternalInput").ap()
    y = nc.dram_tensor("y", [S, D], F32, kind="ExternalOutput").ap()

    dbg = None
    if debug:
        dbg = {
            "cs_dump": nc.dram_tensor("cs_dump", [2, 128, S], BF16, kind="ExternalOutput").ap(),
            "qk_dump": nc.dram_tensor("qk_dump", [128, 8, S], BF16, kind="ExternalOutput").ap(),
            "vp_dump": nc.dram_tensor("vp_dump", [128, NKC, HPC * (DK + 1)], BF16, kind="ExternalOutput").ap(),
            "heads_dump": nc.dram_tensor("heads_dump", [128, 4, S], BF16, kind="ExternalOutput").ap(),
            "recip_dump": nc.dram_tensor("recip_dump", [NSC, 2, 512], F32, kind="ExternalOutput").ap(),
            "rb_dump": nc.dram_tensor("rb_dump", [NSC, 2, DK, 512], F32, kind="ExternalOutput").ap(),
        }

    with tile.TileContext(nc) as tc:
        _emit(nc, tc, xT, wqT, wkT, wvT, woT, pos, invf_in, y, dbg)

    nc.compile()
    return nc


def _emit(nc, tc, xT, wqT, wkT, wvT, woT, pos, invf_in, y, dbg=None):
    import contextlib

    ctx = contextlib.ExitStack()
    with ctx:
        persist = ctx.enter_context(tc.tile_pool(name="persist", bufs=1))
        ones_col = persist.tile([128, 1], BF16)
        nc.vector.memset(ones_col, 1.0)
        identity = persist.tile([128, 128], BF16)
        make_identity(nc, identity)
        # P_swap: swap 32-row blocks within each 64-block (rows 0<->32, 64<->96)
        p_swap = persist.tile([128, 128], BF16)
        for blk in range(4):
            src = (blk ^ 1) * 32
            nc.sync.dma_start(out=p_swap[blk * 32:(blk + 1) * 32, :],
                              in_=identity[src:src + 32, :])

        # cbig/sbig [128, S] bf16: 32-row blocks [cos;cos] and [-sin;sin],
        # replicated to rows 64-127, so RoPE on a [128, s] slice of Q^T/K^T is
        #   q' = q * cbig + (P_swap @ q) * sbig
        cs_pool = ctx.enter_context(tc.tile_pool(name="cs", bufs=1))
        cbig = cs_pool.tile([128, S], BF16)
        sbig = cs_pool.tile([128, S], BF16)

        # ---------------- persistent data tiles ----------------
        qkT_pool = ctx.enter_context(tc.tile_pool(name="qkT", bufs=1))
        qkT = qkT_pool.tile([128, 8, S], BF16)       # q units 0-3, k units 4-7
        vp_pool = ctx.enter_context(tc.tile_pool(name="vp", bufs=1))
        vp = vp_pool.tile([128, NKC, HPC * (DK + 1)], BF16)
        vp_heads = vp.rearrange("p s (h c) -> p s h c", h=HPC)
        nc.scalar.copy(vp_heads[:, :, :, DK:DK + 1],
                       ones_col.to_broadcast((128, NKC, HPC, 1)))

        w_pool = ctx.enter_context(tc.tile_pool(name="w", bufs=1))
        w_qk = w_pool.tile([128, 2, D // 128, DPC], F32R)
        wv_t = w_pool.tile([128, D // 128, DPC], F32R)

        xts_pool = ctx.enter_context(tc.tile_pool(name="xts", bufs=2))
        rope_pool = ctx.enter_context(tc.tile_pool(name="rope", bufs=2))
        e_pool = ctx.enter_context(tc.tile_pool(name="expp", bufs=4))

        # PSUM: 16 KiB/partition = 8 banks, fully budgeted:
        #   sc: [128,1024] f32 x2  (4 banks) score pairs
        #   wk: [128,512]  f32 x2  (2 banks) proj chains / RoPE swap / o_proj
        #   o:  [65,512]   f32 x2  (2 banks) AV accumulators
        ps = ctx.enter_context(tc.tile_pool(name="ps", bufs=1, space="PSUM"))

        # ---------------- rotary tables (two 1024-col halves) --------------
        # DMA priority: table inputs and x chunk 0 go first on the sync queue
        # (each big transfer occupies its queue ~1.7us; the weight stream
        # follows, arriving just in time to pace the first chains).
        tbl = tc.alloc_tile_pool(name="tbl", bufs=1)
        posi = tbl.tile([1, S], I32)
        nc.sync.dma_start(out=posi, in_=pos.unsqueeze(0))
        # invf[r] = theta^(-2r/DK) = exp(-r * ln(theta)/HALF), built on device
        # from a partition iota (a [HALF,1] DMA would be a 32-descriptor
        # partition scatter that stalls the sync queue for ~7us)
        iota_c = tbl.tile([HALF, 1], I32)
        nc.gpsimd.iota(iota_c, pattern=[[0, 1]], base=0, channel_multiplier=1)
        iota_f = tbl.tile([HALF, 1], F32)
        nc.vector.tensor_copy(iota_f, iota_c)
        invf_c = tbl.tile([HALF, 1], F32)
        nc.scalar.activation(invf_c, iota_f, AF.Exp,
                             scale=float(-np.log(THETA) / HALF))
        xts0 = xts_pool.tile([128, D // 128, 512], F32R, name="xts")
        for dc in range(D // 128):
            eng = nc.sync if (dc % 2 == 0) else nc.scalar
            eng.dma_start(out=xts0[:, dc, :],
                          in_=xT.bitcast(F32R)[dc * 128:(dc + 1) * 128, 0:512])
        xts_tiles = {0: xts0}
        for dc in range(D // 128):
            for qk_idx, w_dram in ((0, wqT), (1, wkT)):
                eng = nc.sync if (dc % 2 == 0) else nc.scalar
                eng.dma_start(out=w_qk[:, qk_idx, dc, :],
                              in_=w_dram.bitcast(F32R)[dc * 128:(dc + 1) * 128, :])
            eng = nc.scalar if (dc % 2 == 0) else nc.sync
            eng.dma_start(out=wv_t[:, dc, :],
                          in_=wvT.bitcast(F32R)[dc * 128:(dc + 1) * 128, :])
        for j in range(2):
            sl = bass.ts(j, 1024)
            posf = tbl.tile([1, 1024], F32, name="posf")
            nc.vector.tensor_copy(posf, posi[:, sl])
            posb = tbl.tile([HALF, 1024], F32, name="posb")
            nc.gpsimd.partition_broadcast(posb, posf)
            ang = tbl.tile([HALF, 1024], F32, name="ang")
            nc.vector.tensor_scalar_mul(ang, posb, invf_c)
            k_i = tbl.tile([HALF, 1024], I32, name="k_i")
            nc.scalar.activation(k_i, ang, AF.Copy, scale=float(1.0 / TWO_PI))
            k_f = tbl.tile([HALF, 1024], F32, name="k_f")
            nc.vector.tensor_copy(k_f, k_i)
            ang_red = tbl.tile([HALF, 1024], F32, name="ang_red")
            nc.vector.cody_waite_cascade(ang_red, ang, k_f, _CW_C1, _CW_C2, _CW_C3)
            sin_arg = tbl.tile([HALF, 1024], F32, name="sin_arg")
            cos_arg = tbl.tile([HALF, 1024], F32, name="cos_arg")
            nc.vector.add_range_wrap(sin_arg, ang_red, 0.0, float(np.pi), TWO_PI)
            nc.vector.add_range_wrap(cos_arg, ang_red, float(np.pi / 2),
                                     float(np.pi), TWO_PI)
            nc.scalar.activation(cbig[0:HALF, sl], cos_arg, AF.Sin)
            s_pos = tbl.tile([HALF, 1024], F32, name="s_pos")
            nc.scalar.activation(s_pos, sin_arg, AF.Sin)
            nc.scalar.mul(sbig[0:HALF, sl], s_pos, -1.0)
            nc.vector.tensor_copy(sbig[HALF:2 * HALF, sl], s_pos)
            nc.sync.dma_start(out=cbig[HALF:2 * HALF, sl], in_=cbig[0:HALF, sl])
            nc.sync.dma_start(out=cbig[64:128, sl], in_=cbig[0:64, sl])
            nc.sync.dma_start(out=sbig[64:128, sl], in_=sbig[0:64, sl])
        tbl.release()

        # pools whose SBUF space reuses the (released) table scratch
        heads_pool = ctx.enter_context(tc.tile_pool(name="heads", bufs=1))
        heads_t = heads_pool.tile([128, DPC // 128, S], BF16)
        wo_pool = ctx.enter_context(tc.tile_pool(name="wo", bufs=1))
        wo_t = wo_pool.tile([128, DPC // 128, D], BF16)
        for dc in range(DPC // 128):
            eng = nc.sync if (dc % 2 == 0) else nc.scalar
            eng.dma_start(out=wo_t[:, dc, :],
                          in_=woT[dc * 128:(dc + 1) * 128, :])
        norm_pool = ctx.enter_context(tc.tile_pool(name="norm", bufs=2))
        y_pool = ctx.enter_context(tc.tile_pool(name="yout", bufs=2))

        # ---------------- emission helpers ----------------
        deferred = []           # RoPE tails, emitted one filler late so the
                                # swap matmul never stalls on the qt_sb cast

        def flush_deferred(n=1):
            for _ in range(min(n, len(deferred))):
                deferred.pop(0)()

        def emit_x_load(sc):
            xts_t = xts_pool.tile([128, D // 128, 512], F32R, name="xts")
            for dc in range(D // 128):
                nc.sync.dma_start(
                    out=xts_t[:, dc, :],
                    in_=xT.bitcast(F32R)[dc * 128:(dc + 1) * 128, bass.ts(sc, 512)])
            xts_tiles[sc] = xts_t

        def proj_unit(sc, qk_idx, et):
            # one 128-dim tile of the Q or K projection for chunk sc, + RoPE
            ssl = bass.ts(sc, 512)
            xts_t = xts_tiles[sc]
            pt = ps.tile([128, 512], F32, name="wk", bufs=2)
            for dc in range(D // 128):
                nc.tensor.matmul(pt, w_qk[:, qk_idx, dc, bass.ts(et, 128)],
                                 xts_t[:, dc, :],
                                 start=(dc == 0), stop=(dc == D // 128 - 1))
            qt_sb = rope_pool.tile([128, 512], BF16, name="qt_sb")
            nc.vector.tensor_copy(qt_sb, pt)

            def tail(qt_sb=qt_sb, ssl=ssl, u=qk_idx * 4 + et):
                sw = ps.tile([128, 512], F32, name="wk", bufs=2)
                nc.tensor.matmul(sw, p_swap, qt_sb, start=True, stop=True)
                g1 = rope_pool.tile([128, 512], BF16, name="g1")
                nc.vector.tensor_mul(g1, qt_sb, cbig[:, ssl])
                d1 = rope_pool.tile([128, 512], BF16, name="d1")
                nc.vector.tensor_mul(d1, sw, sbig[:, ssl])
                nc.vector.tensor_add(qkT[:, u, ssl], g1, d1)

            deferred.append(tail)
            if len(deferred) > 1:
                flush_deferred(1)

        def v_unit(sc, st4):
            xts_t = xts_tiles[sc]
            pv = ps.tile([128, 512], F32, name="wk", bufs=2)
            for dc in range(D // 128):
                nc.tensor.matmul(pv, xts_t[:, dc, bass.ts(st4, 128)],
                                 wv_t[:, dc, :],
                                 start=(dc == 0), stop=(dc == D // 128 - 1))
            nc.vector.tensor_copy(vp_heads[:, sc * 4 + st4, :, 0:DK],
                                  pv.rearrange("p (h c) -> p h c", h=HPC))
            flush_deferred(1)

        def o_chain(qc, st4, nb):
            st = qc * 4 + st4
            py = ps.tile([128, 512], F32, name="wk", bufs=2)
            for dc in range(DPC // 128):
                nc.tensor.matmul(py, heads_t[:, dc, bass.ts(st, 128)],
                                 wo_t[:, dc, bass.ts(nb, 512)],
                                 start=(dc == 0), stop=(dc == DPC // 128 - 1))
            y_sb = y_pool.tile([128, 512], F32, name="y_sb")
            nc.vector.tensor_copy(y_sb, py)
            nc.sync.dma_start(out=y[st * 128:(st + 1) * 128, bass.ts(nb, 512)],
                              in_=y_sb)
            flush_deferred(1)

        def attention(qc, front_f, loop_f, tail_f):
            n_kt = 4 * qc + 4
            # front_f run before the first score pair (covers the RoPE wait at
            # qc=0); loop_f are slotted between pairs at a pace that never
            # starves the exp pipeline (~1 chain per 5 pairs); tail_f are
            # heads_t-independent and run during the final norm drain.
            for f in front_f:
                f()
            pair_total = 4 * n_kt
            stride = 5 if len(loop_f) * 5 <= pair_total else max(
                1, pair_total // max(1, len(loop_f)))
            state = {"pair": 0, "fi": 0}

            def maybe_filler():
                if (state["fi"] < len(loop_f)
                        and state["pair"] >= (state["fi"] + 1) * stride):
                    loop_f[state["fi"]]()
                    state["fi"] += 1
                state["pair"] += 1

            for hp in range(HPC // 2):
                hA, hB = 2 * hp, 2 * hp + 1
                o_ts = [ps.tile([DK + 1, 512], F32, name="o", bufs=2)
                        for _ in range(2)]

                def emit_scores(kt):
                    diag = (kt // 4 == qc)
                    co = 128 * (kt % 4) if diag else 0
                    n = 512 - co
                    ktsl = bass.ts(kt, 128)
                    qsl = bass.ds(qc * 512 + co, n)
                    sc_t = ps.tile([128, 1024], F32, name="sc", bufs=2)
                    for i, (ro, tp) in enumerate(((0, (0, 0)), (64, (64, 0)))):
                        nc.tensor.matmul(
                            sc_t[:, i * 512:i * 512 + n],
                            qkT[ro:ro + 64, 4 + hp, ktsl],
                            qkT[ro:ro + 64, hp, qsl],
                            start=True, stop=True, tile_position=tp)
                    e_t = e_pool.tile([128, 1024], BF16, name="e_t")
                    if co == 0:
                        nc.scalar.activation(e_t, sc_t, AF.Exp,
                                             scale=float(1.0 / np.sqrt(DK)))
                    else:
                        for i in range(2):
                            nc.scalar.activation(
                                e_t[:, i * 512:i * 512 + n],
                                sc_t[:, i * 512:i * 512 + n], AF.Exp,
                                scale=float(1.0 / np.sqrt(DK)))
                    if diag:
                        for i in range(2):
                            nc.gpsimd.affine_select(
                                out=e_t[:, i * 512:i * 512 + 128],
                                in_=e_t[:, i * 512:i * 512 + 128],
                                pattern=[[1, 128]], base=0, channel_multiplier=-1,
                                compare_op=mybir.AluOpType.is_ge, fill=0.0)
                    return e_t, co, n

                def emit_av(kt, e_t, co, n):
                    for i, h in enumerate((hA, hB)):
                        nc.tensor.matmul(
                            o_ts[i][:, co:512],
                            vp[:, kt, h * (DK + 1):(h + 1) * (DK + 1)],
                            e_t[:, i * 512:i * 512 + n],
                            start=(kt == 0), stop=(kt == n_kt - 1))

                maybe_filler()
                pend = emit_scores(0)
                for kt in range(1, n_kt):
                    maybe_filler()
                    e = emit_scores(kt)
                    emit_av(kt - 1, *pend)
                    pend = e
                emit_av(n_kt - 1, *pend)
                maybe_filler()

                # drain accumulators to SBUF fast (frees the PSUM "o" slots),
                # then normalize by the ones-column denominator. The very last
                # block skips the staging copy and reads PSUM directly -- its
                # slots need no recycling and the drain is the program tail.
                last_blk = (qc == NSC - 1 and hp == HPC // 2 - 1)
                obs = []
                for i in range(2):
                    if last_blk:
                        obs.append(o_ts[i])
                    else:
                        ob = norm_pool.tile([DK + 1, 512], F32, name="ob")
                        nc.vector.tensor_copy(ob, o_ts[i])
                        obs.append(ob)
                rbs = []
                for i in range(2):
                    dsb = norm_pool.tile([1, 512], F32, name="dsb")
                    nc.vector.tensor_copy(dsb, obs[i][DK:DK + 1, :])
                    recip = norm_pool.tile([1, 512], F32, name="recip")
                    nc.vector.reciprocal_approx_fast(recip, dsb)
                    rb = norm_pool.tile([DK, 512], F32, name="rb")
                    nc.gpsimd.partition_broadcast(rb, recip)
                    rbs.append(rb)
                    if dbg is not None and hp == 0:
                        nc.sync.dma_start(out=dbg["recip_dump"][qc, i].unsqueeze(0), in_=recip)
                        nc.sync.dma_start(out=dbg["rb_dump"][qc, i], in_=rb)
                nc.vector.tensor_mul(
                    heads_t[0:DK, hp, bass.ts(qc, 512)], obs[0][0:DK, :], rbs[0])
                hn = norm_pool.tile([DK, 512], BF16, name="hn")
                nc.vector.tensor_mul(hn, obs[1][0:DK, :], rbs[1])
                nc.sync.dma_start(
                    out=heads_t[DK:128, hp, bass.ts(qc, 512)], in_=hn)

            for f in tail_f:
                f()

        # ---------------- fused schedule ----------------
        def chunk_fillers(sc):
            out = []
            for qk_idx in (0, 1):
                for et in range(4):
                    out.append(lambda s=sc, q=qk_idx, e=et: proj_unit(s, q, e))
            for st4 in range(4):
                out.append(lambda s=sc, t=st4: v_unit(s, t))
            return out

        def oproj_fillers(qc):
            return [lambda q=qc, t=st4, n=nb: o_chain(q, t, n)
                    for st4 in range(4) for nb in range(2)]

        for f in chunk_fillers(0):   # prologue: chunk-0 projections straight
            f()
        for sc in range(NSC):
            front_f, loop_f, tail_f = [], [], []
            if sc + 1 < NSC:
                emit_x_load(sc + 1)
                proj_f = chunk_fillers(sc + 1)
                if sc == 0:
                    front_f = proj_f[:5]
                    proj_f = proj_f[5:]
                loop_f += proj_f[:len(proj_f) - 2]
                tail_f += proj_f[len(proj_f) - 2:]
            if sc > 0:
                # o_proj of the previous chunk leads the loop so it is
                # emitted before this chunk's heads_t writes (avoids false
                # write->read ordering on the coarse heads_t dependency)
                loop_f = oproj_fillers(sc - 1) + loop_f
            attention(sc, front_f, loop_f, tail_f)
        flush_deferred(99)
        # epilogue: last chunk's o_proj, software-pipelined 2 deep so the
        # dc0-2 matmuls (which only need already-normalized head pairs) run
        # while the final pair's normalization drains; only dc3 waits on it.
        epi = [(NSC - 1, st4, nb) for st4 in range(4) for nb in range(2)]
        open_chains = []

        def epi_start(qc, st4, nb):
            st = qc * 4 + st4
            py = ps.tile([128, 512], F32, name="wk", bufs=2)
            for dc in range(DPC // 128 - 1):
                nc.tensor.matmul(py, heads_t[:, dc, bass.ts(st, 128)],
                                 wo_t[:, dc, bass.ts(nb, 512)],
                                 start=(dc == 0), stop=False)
            return py, st, nb

        def epi_finish(py, st, nb):
            dc = DPC // 128 - 1
            nc.tensor.matmul(py, heads_t[:, dc, bass.ts(st, 128)],
                             wo_t[:, dc, bass.ts(nb, 512)],
                             start=False, stop=True)
            y_sb = y_pool.tile([128, 512], F32, name="y_sb")
            nc.vector.tensor_copy(y_sb, py)
            nc.sync.dma_start(out=y[st * 128:(st + 1) * 128, bass.ts(nb, 512)],
                              in_=y_sb)

        for k, c in enumerate(epi):
            if len(open_chains) == 2:
                epi_finish(*open_chains.pop(0))
            open_chains.append(epi_start(*c))
        while open_chains:
            epi_finish(*open_chains.pop(0))

        if dbg is not None:
            nc.sync.dma_start(out=dbg["cs_dump"][0], in_=cbig)
            nc.sync.dma_start(out=dbg["cs_dump"][1], in_=sbig)
            nc.sync.dma_start(out=dbg["qk_dump"], in_=qkT)
            nc.sync.dma_start(out=dbg["vp_dump"], in_=vp)
            nc.sync.dma_start(out=dbg["heads_dump"], in_=heads_t)


def _host_inv_freq():
    inv = 1.0 / (THETA ** (np.arange(HALF, dtype=np.float64) * 2.0 / DK))
    return inv.astype(np.float32)


_program_cache = None


def _get_program():
    global _program_cache
    if _program_cache is None:
        _program_cache = _build_program()
    return _program_cache


# dk permutation: evens then odds within each head's 64 dims
_PERM64 = np.concatenate([np.arange(0, DK, 2), np.arange(1, DK, 2)])


def _make_in_maps(x, Wq, Wk, Wv, Wo, pos_np):
    import ml_dtypes
    invf_np = _host_inv_freq()
    in_maps = []
    for c in range(N_CORES):
        b, hg = c // 2, c % 2
        rows = hg * DPC + np.concatenate(
            [h * DK + _PERM64 for h in range(HPC)])
        in_maps.append({
            "xT": np.ascontiguousarray(x[b].T),
            "wqT": np.ascontiguousarray(Wq[rows, :].T),
            "wkT": np.ascontiguousarray(Wk[rows, :].T),
            "wvT": np.ascontiguousarray(Wv[hg * DPC:(hg + 1) * DPC, :].T),
            "woT": np.ascontiguousarray(
                Wo[:, hg * DPC:(hg + 1) * DPC].T).astype(ml_dtypes.bfloat16),
            "pos": pos_np,
            "invf": invf_np,
        })
    return in_maps


def kernel(x, Wq, Wk, Wv, Wo, token_positions):
    x = np.asarray(x, dtype=np.float32)
    Wq = np.asarray(Wq, dtype=np.float32)
    Wk = np.asarray(Wk, dtype=np.float32)
    Wv = np.asarray(Wv, dtype=np.float32)
    Wo = np.asarray(Wo, dtype=np.float32)
    pos_np = np.ascontiguousarray(np.asarray(token_positions, dtype=np.int32))

    nc = _get_program()
    in_maps = _make_in_maps(x, Wq, Wk, Wv, Wo, pos_np)
    res = run_bass_kernel_spmd(nc, in_maps, list(range(N_CORES)))
    out = np.empty((B, S, D), dtype=np.float32)
    for b in range(B):
        out[b] = res.results[2 * b]["y"] + res.results[2 * b + 1]["y"]
    return out


# revision 14
# speedup vs baseline: 1.0299x; 1.0299x over previous
"""Causal multi-head self-attention with RoPE on 8 Trainium2 NeuronCores.

Sharding: data-parallel over batch (B=4 -> 2 cores per batch) x tensor-parallel
over heads (16 heads -> 8 per core). Each core computes q/k/v projections for
its 8 heads, RoPE, causal attention, and a partial o_proj; the host sums the
two partial o_proj outputs per batch.

Structure (v3):
  - One fused pipeline: per 512-query chunk sc, the Q/K/V projection chains of
    chunk sc+1 and the o_proj chains of chunk sc-1 are interleaved as PE
    "filler" work inside attention(sc)'s scores/AV stream, so the tensor
    engine never idles at phase boundaries and stays at full p-state clock.
    Fillers are front-loaded at sc=0 (covers the RoPE(0) wait) and reserved
    at each chunk's end (covers the normalization drain).
  - bf16 for qkT/vp/e_t/heads/wo (1 cyc/row at any N, incl. the short
    diagonal tiles that f32r runs at 4 cyc/row); x/Wq/Wk/Wv stay f32r so the
    softmax logits keep ~1e-2 absolute accuracy. The tile_position-packed
    score pairs overlap in the PE row groups (2nd matmul of a pair is ~free).
  - RoPE swap is a PE matmul against a bf16 block-swap permutation; its
    emission is deferred one filler so the PE never waits on the PSUM->SBUF
    cast of its input. (DMA-based swap costs ~600ns of descriptor-gen
    dispatch per transfer -- far worse than the 213ns matmul.)
  - Rotary tables are built on device (Cody-Waite range reduction, two
    1024-col halves); the angle outer product runs on gpsimd+DVE
    (partition_broadcast + tensor_scalar_mul), keeping the cold-clock PE
    out of the table critical path. pos*invf needs exact f32 here --
    f32r rounding of invf turns into O(1) rad angle error at pos ~ 2e3.
  - PSUM budget (8 banks): score pairs [128,1024] x2 (4 banks), work tiles
    [128,512] x2 (proj chains / RoPE swap / o_proj, 2 banks), AV
    accumulators [65,512] x2 (2 banks, drained to SBUF by DVE immediately).
  - exp runs on the scalar engine straight out of PSUM over the packed
    head-pair tile (the attention cadence is exp-bound at ~1.1us/pair; PE
    fillers absorb the slack). Causal masking of the diagonal 128-col block
    is a gpsimd affine_select on e_t.
  - Normalization: DVE reciprocal (via a partition-0 staging copy -- the
    approx-reciprocal sequence cannot take a partition-offset input),
    gpsimd broadcast, DVE multiply writing heads_t directly for the even
    head and via one DMA for the odd head (cross-partition move).
"""

import sys

sys.path.insert(0, "/opt/trn_rl_repo")

import numpy as np

import concourse.bass as bass
import concourse.tile as tile
from concourse import bacc, mybir
from concourse.bass_utils import run_bass_kernel_spmd
from concourse.masks import make_identity

B, S, D, H = 4, 2048, 1024, 16
DK = D // H            # 64
HPC = H // 2           # 8 heads per core
DPC = HPC * DK         # 512 head dims per core
N_CORES = 8
HALF = DK // 2         # 32 rotary pairs
THETA = 10000.0
NKC = S // 128         # 16 key tiles
NSC = S // 512         # 4 query/proj chunks

AF = mybir.ActivationFunctionType
F32 = mybir.dt.float32
F32R = mybir.dt.float32r
BF16 = mybir.dt.bfloat16
I32 = mybir.dt.int32

TWO_PI = 2.0 * np.pi
# 3-term Cody-Waite split of 2*pi (c1/c2 have short mantissas so k*c is exact)
_CW_C1 = 6.28125
_CW_C2 = float(np.float32(9.67025756835937500e-4))
_CW_C3 = float(TWO_PI - _CW_C1 - np.float32(9.67025756835937500e-4))


def _build_program(debug=False):
    nc = bacc.Bacc("TRN2", target_bir_lowering=False, debug=False)

    xT = nc.dram_tensor("xT", [D, S], F32, kind="ExternalInput").ap()
    wqT = nc.dram_tensor("wqT", [D, DPC], F32, kind="ExternalInput").ap()
    wkT = nc.dram_tensor("wkT", [D, DPC], F32, kind="ExternalInput").ap()
    wvT = nc.dram_tensor("wvT", [D, DPC], F32, kind="ExternalInput").ap()
    woT = nc.dram_tensor("woT", [DPC, D], BF16, kind="ExternalInput").ap()
    pos = nc.dram_tensor("pos", [S], I32, kind="ExternalInput").ap()
    invf_in = nc.dram_tensor("invf", [HALF], F32, kind="ExternalInput").ap()
    y = nc.dram_tensor("y", [S, D], F32, kind="ExternalOutput").ap()

    dbg = None
    if debug:
        dbg = {
            "cs_dump": nc.dram_tensor("cs_dump", [2, 128, S], BF16, kind="ExternalOutput").ap(),
            "qk_dump": nc.dram_tensor("qk_dump", [128, 8, S], BF16, kind="ExternalOutput").ap(),
            "vp_dump": nc.dram_tensor("vp_dump", [128, NKC, HPC * (DK + 1)], BF16, kind="ExternalOutput").ap(),
            "heads_dump": nc.dram_tensor("heads_dump", [128, 4, S], BF16, kind="ExternalOutput").ap(),
            "recip_dump": nc.dram_tensor("recip_dump", [NSC, 2, 512], F32, kind="ExternalOutput").ap(),
            "rb_dump": nc.dram_tensor("rb_dump", [NSC, 2, DK, 512], F32, kind="ExternalOutput").ap(),
        }

    with tile.TileContext(nc) as tc:
        _emit(nc, tc, xT, wqT, wkT, wvT, woT, pos, invf_in, y, dbg)

    nc.compile()
    return nc


def _emit(nc, tc, xT, wqT, wkT, wvT, woT, pos, invf_in, y, dbg=None):
    import contextlib

    ctx = contextlib.ExitStack()
    with ctx:
        persist = ctx.enter_context(tc.tile_pool(name="persist", bufs=1))
        ones_col = persist.tile([128, 1], BF16)
        nc.vector.memset(ones_col, 1.0)
        identity = persist.tile([128, 128], BF16)
        make_identity(nc, identity)
        # P_swap: swap 32-row blocks within each 64-block (rows 0<->32, 64<->96)
        p_swap = persist.tile([128, 128], BF16)
        for blk in range(4):
            src = (blk ^ 1) * 32
            nc.sync.dma_start(out=p_swap[blk * 32:(blk + 1) * 32, :],
                              in_=identity[src:src + 32, :])

        # cbig/sbig [128, S] bf16: 32-row blocks [cos;cos] and [-sin;sin],
        # replicated to rows 64-127, so RoPE on a [128, s] slice of Q^T/K^T is
        #   q' = q * cbig + (P_swap @ q) * sbig
        cs_pool = ctx.enter_context(tc.tile_pool(name="cs", bufs=1))
        cbig = cs_pool.tile([128, S], BF16)
        sbig = cs_pool.tile([128, S], BF16)

        # ---------------- persistent data tiles ----------------
        qkT_pool = ctx.enter_context(tc.tile_pool(name="qkT", bufs=1))
        qkT = qkT_pool.tile([128, 8, S], BF16)       # q units 0-3, k units 4-7
        vp_pool = ctx.enter_context(tc.tile_pool(name="vp", bufs=1))
        vp = vp_pool.tile([128, NKC, HPC * (DK + 1)], BF16)
        vp_heads = vp.rearrange("p s (h c) -> p s h c", h=HPC)
        nc.scalar.copy(vp_heads[:, :, :, DK:DK + 1],
                       ones_col.to_broadcast((128, NKC, HPC, 1)))

        w_pool = ctx.enter_context(tc.tile_pool(name="w", bufs=1))
        w_qk = w_pool.tile([128, 2, D // 128, DPC], F32R)
        wv_t = w_pool.tile([128, D // 128, DPC], F32R)

        xts_pool = ctx.enter_context(tc.tile_pool(name="xts", bufs=2))
        rope_pool = ctx.enter_context(tc.tile_pool(name="rope", bufs=2))
        e_pool = ctx.enter_context(tc.tile_pool(name="expp", bufs=4))

        # PSUM: 16 KiB/partition = 8 banks, fully budgeted:
        #   sc: [128,1024] f32 x2  (4 banks) score pairs
        #   wk: [128,512]  f32 x2  (2 banks) proj chains / RoPE swap / o_proj
        #   o:  [65,512]   f32 x2  (2 banks) AV accumulators
        ps = ctx.enter_context(tc.tile_pool(name="ps", bufs=1, space="PSUM"))

        # ---------------- rotary tables (two 1024-col halves) --------------
        # DMA priority: table inputs and x chunk 0 go first on the sync queue
        # (each big transfer occupies its queue ~1.7us; the weight stream
        # follows, arriving just in time to pace the first chains).
        tbl = tc.alloc_tile_pool(name="tbl", bufs=1)
        posi = tbl.tile([1, S], I32)
        nc.sync.dma_start(out=posi, in_=pos.unsqueeze(0))
        # invf[r] = theta^(-2r/DK) = exp(-r * ln(theta)/HALF), built on device
        # from a partition iota (a [HALF,1] DMA would be a 32-descriptor
        # partition scatter that stalls the sync queue for ~7us)
        iota_c = tbl.tile([HALF, 1], I32)
        nc.gpsimd.iota(iota_c, pattern=[[0, 1]], base=0, channel_multiplier=1)
        iota_f = tbl.tile([HALF, 1], F32)
        nc.vector.tensor_copy(iota_f, iota_c)
        invf_c = tbl.tile([HALF, 1], F32)
        nc.scalar.activation(invf_c, iota_f, AF.Exp,
                             scale=float(-np.log(THETA) / HALF))
        xts0 = xts_pool.tile([128, D // 128, 512], F32R, name="xts")
        for dc in range(D // 128):
            eng = nc.sync if (dc % 2 == 0) else nc.scalar
            eng.dma_start(out=xts0[:, dc, :],
                          in_=xT.bitcast(F32R)[dc * 128:(dc + 1) * 128, 0:512])
        xts_tiles = {0: xts0}
        for qk_idx, w_dram in ((0, wqT), (1, wkT)):
            for dc in range(D // 128):
                eng = nc.sync if (dc % 2 == 0) else nc.scalar
                eng.dma_start(out=w_qk[:, qk_idx, dc, :],
                              in_=w_dram.bitcast(F32R)[dc * 128:(dc + 1) * 128, :])
        for dc in range(D // 128):
            eng = nc.scalar if (dc % 2 == 0) else nc.sync
            eng.dma_start(out=wv_t[:, dc, :],
                          in_=wvT.bitcast(F32R)[dc * 128:(dc + 1) * 128, :])
        for j in range(2):
            sl = bass.ts(j, 1024)
            posf = tbl.tile([1, 1024], F32, name="posf")
            nc.vector.tensor_copy(posf, posi[:, sl])
            posb = tbl.tile([HALF, 1024], F32, name="posb")
            nc.gpsimd.partition_broadcast(posb, posf)
            ang = tbl.tile([HALF, 1024], F32, name="ang")
            nc.vector.tensor_scalar_mul(ang, posb, invf_c)
            k_i = tbl.tile([HALF, 1024], I32, name="k_i")
            nc.scalar.activation(k_i, ang, AF.Copy, scale=float(1.0 / TWO_PI))
            k_f = tbl.tile([HALF, 1024], F32, name="k_f")
            nc.vector.tensor_copy(k_f, k_i)
            ang_red = tbl.tile([HALF, 1024], F32, name="ang_red")
            nc.vector.cody_waite_cascade(ang_red, ang, k_f, _CW_C1, _CW_C2, _CW_C3)
            sin_arg = tbl.tile([HALF, 1024], F32, name="sin_arg")
            cos_arg = tbl.tile([HALF, 1024], F32, name="cos_arg")
            nc.vector.add_range_wrap(sin_arg, ang_red, 0.0, float(np.pi), TWO_PI)
            nc.vector.add_range_wrap(cos_arg, ang_red, float(np.pi / 2),
                                     float(np.pi), TWO_PI)
            nc.scalar.activation(cbig[0:HALF, sl], cos_arg, AF.Sin)
            s_pos = tbl.tile([HALF, 1024], F32, name="s_pos")
            nc.scalar.activation(s_pos, sin_arg, AF.Sin)
            nc.scalar.mul(sbig[0:HALF, sl], s_pos, -1.0)
            nc.vector.tensor_copy(sbig[HALF:2 * HALF, sl], s_pos)
            nc.sync.dma_start(out=cbig[HALF:2 * HALF, sl], in_=cbig[0:HALF, sl])
            nc.sync.dma_start(out=cbig[64:128, sl], in_=cbig[0:64, sl])
            nc.sync.dma_start(out=sbig[64:128, sl], in_=sbig[0:64, sl])
        tbl.release()

        # pools whose SBUF space reuses the (released) table scratch
        heads_pool = ctx.enter_context(tc.tile_pool(name="heads", bufs=1))
        # one tile per head-pair: keeps o_proj reads dependent only on the
        # pairs they actually consume (a single [128,4,S] tile made every
        # o_proj chain wait for the newest normalization write)
        heads_hp = [heads_pool.tile([128, S], BF16, name=f"h{i}")
                    for i in range(DPC // 128)]
        wo_pool = ctx.enter_context(tc.tile_pool(name="wo", bufs=1))
        wo_t = wo_pool.tile([128, DPC // 128, D], BF16)
        for dc in range(DPC // 128):
            eng = nc.sync if (dc % 2 == 0) else nc.scalar
            eng.dma_start(out=wo_t[:, dc, :],
                          in_=woT[dc * 128:(dc + 1) * 128, :])
        norm_pool = ctx.enter_context(tc.tile_pool(name="norm", bufs=2))
        y_pool = ctx.enter_context(tc.tile_pool(name="yout", bufs=2))

        # ---------------- emission helpers ----------------
        deferred = []           # RoPE tails, emitted one filler late so the
                                # swap matmul never stalls on the qt_sb cast

        def flush_deferred(n=1):
            for _ in range(min(n, len(deferred))):
                deferred.pop(0)()

        def emit_x_load(sc):
            xts_t = xts_pool.tile([128, D // 128, 512], F32R, name="xts")
            for dc in range(D // 128):
                nc.sync.dma_start(
                    out=xts_t[:, dc, :],
                    in_=xT.bitcast(F32R)[dc * 128:(dc + 1) * 128, bass.ts(sc, 512)])
            xts_tiles[sc] = xts_t

        def proj_unit(sc, qk_idx, et):
            # one 128-dim tile of the Q or K projection for chunk sc, + RoPE
            ssl = bass.ts(sc, 512)
            xts_t = xts_tiles[sc]
            pt = ps.tile([128, 512], F32, name="wk", bufs=2)
            for dc in range(D // 128):
                nc.tensor.matmul(pt, w_qk[:, qk_idx, dc, bass.ts(et, 128)],
                                 xts_t[:, dc, :],
                                 start=(dc == 0), stop=(dc == D // 128 - 1))
            qt_sb = rope_pool.tile([128, 512], BF16, name="qt_sb")
            nc.vector.tensor_copy(qt_sb, pt)

            def tail(qt_sb=qt_sb, ssl=ssl, u=qk_idx * 4 + et):
                sw = ps.tile([128, 512], F32, name="wk", bufs=2)
                nc.tensor.matmul(sw, p_swap, qt_sb, start=True, stop=True)
                g1 = rope_pool.tile([128, 512], BF16, name="g1")
                nc.vector.tensor_mul(g1, qt_sb, cbig[:, ssl])
                d1 = rope_pool.tile([128, 512], BF16, name="d1")
                nc.vector.tensor_mul(d1, sw, sbig[:, ssl])
                nc.vector.tensor_add(qkT[:, u, ssl], g1, d1)

            deferred.append(tail)
            if len(deferred) > 1:
                flush_deferred(1)

        def v_unit(sc, st4):
            xts_t = xts_tiles[sc]
            pv = ps.tile([128, 512], F32, name="wk", bufs=2)
            for dc in range(D // 128):
                nc.tensor.matmul(pv, xts_t[:, dc, bass.ts(st4, 128)],
                                 wv_t[:, dc, :],
                                 start=(dc == 0), stop=(dc == D // 128 - 1))
            nc.vector.tensor_copy(vp_heads[:, sc * 4 + st4, :, 0:DK],
                                  pv.rearrange("p (h c) -> p h c", h=HPC))
            flush_deferred(1)

        def o_chain(qc, st4, nb):
            st = qc * 4 + st4
            py = ps.tile([128, 512], F32, name="wk", bufs=2)
            for dc in range(DPC // 128):
                nc.tensor.matmul(py, heads_hp[dc][:, bass.ts(st, 128)],
                                 wo_t[:, dc, bass.ts(nb, 512)],
                                 start=(dc == 0), stop=(dc == DPC // 128 - 1))
            y_sb = y_pool.tile([128, 512], F32, name="y_sb")
            nc.vector.tensor_copy(y_sb, py)
            nc.sync.dma_start(out=y[st * 128:(st + 1) * 128, bass.ts(nb, 512)],
                              in_=y_sb)
            flush_deferred(1)

        def attention(qc, front_f, loop_f, tail_f):
            n_kt = 4 * qc + 4
            # front_f run before the first score pair (covers the RoPE wait at
            # qc=0); loop_f are slotted between pairs at a pace that never
            # starves the exp pipeline (~1 chain per 5 pairs); tail_f are
            # heads_t-independent and run during the final norm drain.
            for f in front_f:
                f()
            pair_total = 4 * n_kt
            stride = 5 if len(loop_f) * 5 <= pair_total else max(
                1, pair_total // max(1, len(loop_f)))
            state = {"pair": 0, "fi": 0}

            def maybe_filler():
                if (state["fi"] < len(loop_f)
                        and state["pair"] >= (state["fi"] + 1) * stride):
                    loop_f[state["fi"]]()
                    state["fi"] += 1
                state["pair"] += 1

            for hp in range(HPC // 2):
                hA, hB = 2 * hp, 2 * hp + 1
                o_ts = [ps.tile([DK + 1, 512], F32, name="o", bufs=2)
                        for _ in range(2)]

                def emit_scores(kt):
                    diag = (kt // 4 == qc)
                    co = 128 * (kt % 4) if diag else 0
                    n = 512 - co
                    ktsl = bass.ts(kt, 128)
                    qsl = bass.ds(qc * 512 + co, n)
                    sc_t = ps.tile([128, 1024], F32, name="sc", bufs=2)
                    for i, (ro, tp) in enumerate(((0, (0, 0)), (64, (64, 0)))):
                        nc.tensor.matmul(
                            sc_t[:, i * 512:i * 512 + n],
                            qkT[ro:ro + 64, 4 + hp, ktsl],
                            qkT[ro:ro + 64, hp, qsl],
                            start=True, stop=True, tile_position=tp)
                    e_t = e_pool.tile([128, 1024], BF16, name="e_t")
                    if co == 0:
                        nc.scalar.activation(e_t, sc_t, AF.Exp,
                                             scale=float(1.0 / np.sqrt(DK)))
                    else:
                        for i in range(2):
                            nc.scalar.activation(
                                e_t[:, i * 512:i * 512 + n],
                                sc_t[:, i * 512:i * 512 + n], AF.Exp,
                                scale=float(1.0 / np.sqrt(DK)))
                    if diag:
                        for i in range(2):
                            nc.gpsimd.affine_select(
                                out=e_t[:, i * 512:i * 512 + 128],
                                in_=e_t[:, i * 512:i * 512 + 128],
                                pattern=[[1, 128]], base=0, channel_multiplier=-1,
                                compare_op=mybir.AluOpType.is_ge, fill=0.0)
                    return e_t, co, n

                def emit_av(kt, e_t, co, n):
                    for i, h in enumerate((hA, hB)):
                        nc.tensor.matmul(
                            o_ts[i][:, co:512],
                            vp[:, kt, h * (DK + 1):(h + 1) * (DK + 1)],
                            e_t[:, i * 512:i * 512 + n],
                            start=(kt == 0), stop=(kt == n_kt - 1))

                maybe_filler()
                pend = emit_scores(0)
                for kt in range(1, n_kt):
                    maybe_filler()
                    e = emit_scores(kt)
                    emit_av(kt - 1, *pend)
                    pend = e
                emit_av(n_kt - 1, *pend)
                maybe_filler()

                # drain accumulators to SBUF fast (frees the PSUM "o" slots),
                # then normalize by the ones-column denominator. The very last
                # block skips the staging copy and reads PSUM directly -- its
                # slots need no recycling and the drain is the program tail.
                last_blk = (qc == NSC - 1 and hp == HPC // 2 - 1)
                obs = []
                for i in range(2):
                    if last_blk:
                        obs.append(o_ts[i])
                    else:
                        ob = norm_pool.tile([DK + 1, 512], F32, name="ob")
                        nc.vector.tensor_copy(ob, o_ts[i])
                        obs.append(ob)
                rbs = []
                for i in range(2):
                    dsb = norm_pool.tile([1, 512], F32, name="dsb")
                    nc.vector.tensor_copy(dsb, obs[i][DK:DK + 1, :])
                    recip = norm_pool.tile([1, 512], F32, name="recip")
                    nc.vector.reciprocal_approx_fast(recip, dsb)
                    rb = norm_pool.tile([DK, 512], F32, name="rb")
                    nc.gpsimd.partition_broadcast(rb, recip)
                    rbs.append(rb)
                    if dbg is not None and hp == 0:
                        nc.sync.dma_start(out=dbg["recip_dump"][qc, i].unsqueeze(0), in_=recip)
                        nc.sync.dma_start(out=dbg["rb_dump"][qc, i], in_=rb)
                nc.vector.tensor_mul(
                    heads_hp[hp][0:DK, bass.ts(qc, 512)], obs[0][0:DK, :], rbs[0])
                hn = norm_pool.tile([DK, 512], BF16, name="hn")
                nc.vector.tensor_mul(hn, obs[1][0:DK, :], rbs[1])
                nc.sync.dma_start(
                    out=heads_hp[hp][DK:128, bass.ts(qc, 512)], in_=hn)

            for f in tail_f:
                f()

        # ---------------- fused schedule ----------------
        def chunk_fillers(sc):
            out = []
            for qk_idx in (0, 1):
                for et in range(4):
                    out.append(lambda s=sc, q=qk_idx, e=et: proj_unit(s, q, e))
            for st4 in range(4):
                out.append(lambda s=sc, t=st4: v_unit(s, t))
            return out

        def oproj_fillers(qc):
            return [lambda q=qc, t=st4, n=nb: o_chain(q, t, n)
                    for st4 in range(4) for nb in range(2)]

        for f in chunk_fillers(0):   # prologue: chunk-0 projections straight
            f()
        for sc in range(NSC):
            front_f, loop_f, tail_f = [], [], []
            if sc + 1 < NSC:
                emit_x_load(sc + 1)
                proj_f = chunk_fillers(sc + 1)
                if sc == 0:
                    front_f = proj_f[:5]
                    proj_f = proj_f[5:]
                loop_f += proj_f[:len(proj_f) - 2]
                tail_f += proj_f[len(proj_f) - 2:]
            if sc == NSC - 1:
                # all deferred o_proj chains land in the last window -- it is
                # the most exp-bound one, with enough PE slack to hide them
                for qcp in range(NSC - 1):
                    loop_f += oproj_fillers(qcp)
            attention(sc, front_f, loop_f, tail_f)
        flush_deferred(99)
        # epilogue: last chunk's o_proj, software-pipelined 2 deep so the
        # dc0-2 matmuls (which only need already-normalized head pairs) run
        # while the final pair's normalization drains; only dc3 waits on it.
        epi = [(NSC - 1, st4, nb) for st4 in range(4) for nb in range(2)]
        open_chains = []

        def epi_start(qc, st4, nb):
            st = qc * 4 + st4
            py = ps.tile([128, 512], F32, name="wk", bufs=2)
            for dc in range(DPC // 128 - 1):
                nc.tensor.matmul(py, heads_hp[dc][:, bass.ts(st, 128)],
                                 wo_t[:, dc, bass.ts(nb, 512)],
                                 start=(dc == 0), stop=False)
            return py, st, nb

        def epi_finish(py, st, nb):
            dc = DPC // 128 - 1
            nc.tensor.matmul(py, heads_hp[dc][:, bass.ts(st, 128)],
                             wo_t[:, dc, bass.ts(nb, 512)],
                             start=False, stop=True)
            y_sb = y_pool.tile([128, 512], F32, name="y_sb")
            nc.vector.tensor_copy(y_sb, py)
            nc.sync.dma_start(out=y[st * 128:(st + 1) * 128, bass.ts(nb, 512)],
                              in_=y_sb)

        for k, c in enumerate(epi):
            if len(open_chains) == 2:
                epi_finish(*open_chains.pop(0))
            open_chains.append(epi_start(*c))
        while open_chains:
            epi_finish(*open_chains.pop(0))

        if dbg is not None:
            nc.sync.dma_start(out=dbg["cs_dump"][0], in_=cbig)
            nc.sync.dma_start(out=dbg["cs_dump"][1], in_=sbig)
            nc.sync.dma_start(out=dbg["qk_dump"], in_=qkT)
            nc.sync.dma_start(out=dbg["vp_dump"], in_=vp)
            for i in range(4):
                nc.sync.dma_start(out=dbg["heads_dump"][:, i, :], in_=heads_hp[i])


def _host_inv_freq():
    inv = 1.0 / (THETA ** (np.arange(HALF, dtype=np.float64) * 2.0 / DK))
    return inv.astype(np.float32)


_program_cache = None


def _get_program():
    global _program_cache
    if _program_cache is None:
        _program_cache = _build_program()
    return _program_cache


# dk permutation: evens then odds within each head's 64 dims
_PERM64 = np.concatenate([np.arange(0, DK, 2), np.arange(1, DK, 2)])


def _make_in_maps(x, Wq, Wk, Wv, Wo, pos_np):
    import ml_dtypes
    invf_np = _host_inv_freq()
    in_maps = []
    for c in range(N_CORES):
        b, hg = c // 2, c % 2
        rows = hg * DPC + np.concatenate(
            [h * DK + _PERM64 for h in range(HPC)])
        in_maps.append({
            "xT": np.ascontiguousarray(x[b].T),
            "wqT": np.ascontiguousarray(Wq[rows, :].T),
            "wkT": np.ascontiguousarray(Wk[rows, :].T),
            "wvT": np.ascontiguousarray(Wv[hg * DPC:(hg + 1) * DPC, :].T),
            "woT": np.ascontiguousarray(
                Wo[:, hg * DPC:(hg + 1) * DPC].T).astype(ml_dtypes.bfloat16),
            "pos": pos_np,
            "invf": invf_np,
        })
    return in_maps


def kernel(x, Wq, Wk, Wv, Wo, token_positions):
    x = np.asarray(x, dtype=np.float32)
    Wq = np.asarray(Wq, dtype=np.float32)
    Wk = np.asarray(Wk, dtype=np.float32)
    Wv = np.asarray(Wv, dtype=np.float32)
    Wo = np.asarray(Wo, dtype=np.float32)
    pos_np = np.ascontiguousarray(np.asarray(token_positions, dtype=np.int32))

    nc = _get_program()
    in_maps = _make_in_maps(x, Wq, Wk, Wv, Wo, pos_np)
    res = run_bass_kernel_spmd(nc, in_maps, list(range(N_CORES)))
    out = np.empty((B, S, D), dtype=np.float32)
    for b in range(B):
        out[b] = res.results[2 * b]["y"] + res.results[2 * b + 1]["y"]
    return out
